# revision 29
# baseline (speedup 1.0000x reference)
"""Causal self-attention (B=4, T=2048, C=1024, H=16, rope) on 8 trn2 cores.

Sharding: data-parallel over B (4) x tensor-parallel over heads (2 groups of
8 heads). Core (b, g) computes its batch's Q/K/V for its 8 heads, the full
causal attention for those heads, and a partial output projection
(y_heads @ wp_cols.T). Host sums the two head-group (bf16) partials per batch
and adds the output bias.

Device layout notes:
  - Q^T/K^T are kept as [c_out, t] tiles (partition = head-dim, 2 heads per
    128-partition tile) so QK^T needs no transposes; scores are computed as
    S^T[j, i] tiles (partition = key pos, free = query pos).
  - RoPE: dst = q*C + swap(q)*S is computed as q*C + swap(q*S'') with
    S'' = swap(S), so both multiplies read the projection PSUM directly
    (no raw copy) and the half-swap runs as 4 block DMAs afterwards.
  - Softmax denominators come free from an extra all-ones column appended to
    V (row 64 of the O^T accumulation); no max-subtraction is needed because
    the logits are bounded for this problem scale.
  - Matmul operands are bf16 (full-rate PE path); accumulation stays fp32 in
    PSUM.
  - Emission order pipelines pair-0 projections -> attention(p) under which
    the scheduler back-fills pair p+1 projections and (for p=3) the output
    projection, keeping PE dense below the scalar-engine exp stream.
  - All DRAM operands are host-pre-shuffled so every DMA is contiguous per
    partition (2-8 KB runs); inputs stream on both HWDGE rings in parallel.
"""

import sys

if "/opt/trn_rl_repo" not in sys.path:
    sys.path.insert(0, "/opt/trn_rl_repo")

from contextlib import ExitStack

import numpy as np
import ml_dtypes

import concourse.bass as bass
import concourse.mybir as mybir
from concourse import bacc
from concourse.bass_utils import run_bass_kernel_spmd
from concourse.tile import TileContext

B, T, C = 4, 2048, 1024
H = 16
D = 64
NCORES = 8
CL = C // 2  # per-core c_out (8 heads * 64)
HL = 8  # local heads
F = mybir.dt.float32
FR = mybir.dt.bfloat16  # matmul operand dtype

_NC_CACHE = {}


def _build_nc(with_bias: bool):
    KC = 9 if with_bias else 8  # c_in chunks of 128 (one extra for bias row)
    nc = bacc.Bacc("TRN2", debug=False, num_devices=NCORES)

    xT4 = nc.declare_dram_parameter("xT4", [4, 128, KC, 512], FR, isOutput=False).ap()
    wqT4 = nc.declare_dram_parameter("wqT4", [4, 128, KC, 128], FR, isOutput=False).ap()
    wkT4 = nc.declare_dram_parameter("wkT4", [4, 128, KC, 128], FR, isOutput=False).ap()
    wvT = nc.declare_dram_parameter("wvT", [128, KC, 512], FR, isOutput=False).ap()
    wpT = nc.declare_dram_parameter("wpT", [128, 4, C], FR, isOutput=False).ap()
    onesV = nc.declare_dram_parameter("onesV", [128, 16 * HL], FR, isOutput=False).ap()
    ropeC = nc.declare_dram_parameter("ropeC", [128, T], FR, isOutput=False).ap()
    ropeS2 = nc.declare_dram_parameter("ropeS2", [128, T], FR, isOutput=False).ap()
    dmask = nc.declare_dram_parameter("dmask", [128, 128], FR, isOutput=False).ap()
    esel = nc.declare_dram_parameter("esel", [2, 128], FR, isOutput=False).ap()
    out = nc.declare_dram_parameter("out", [T, C], FR, isOutput=True).ap()

    EXP = mybir.ActivationFunctionType.Exp
    scale = 1.0 / float(np.sqrt(D))

    with TileContext(nc) as tc:
        with ExitStack() as ctx:
            # long-lived SBUF pools
            qk_pool = ctx.enter_context(tc.tile_pool(name="qk", bufs=1))
            v_pool = ctx.enter_context(tc.tile_pool(name="v", bufs=1))
            wpool = ctx.enter_context(tc.tile_pool(name="w", bufs=1))
            xpool = ctx.enter_context(tc.tile_pool(name="x", bufs=1))
            rpool = ctx.enter_context(tc.tile_pool(name="rope", bufs=1))
            c2 = ctx.enter_context(tc.tile_pool(name="c2", bufs=1))
            tpool = ctx.enter_context(tc.tile_pool(name="t1", bufs=2))
            fpool = ctx.enter_context(tc.tile_pool(name="fr", bufs=1))
            # phase-2 SBUF pools
            ppool = ctx.enter_context(tc.tile_pool(name="pt", bufs=3))
            yrawp = ctx.enter_context(tc.tile_pool(name="yraw", bufs=3))
            ytmpp = ctx.enter_context(tc.tile_pool(name="ytmp", bufs=2))
            dpool = ctx.enter_context(tc.tile_pool(name="dd", bufs=2))
            ynp = ctx.enter_context(tc.tile_pool(name="yn", bufs=17))
            osbp = ctx.enter_context(tc.tile_pool(name="osb", bufs=3))
            bcpool = ctx.enter_context(tc.tile_pool(name="bc", bufs=3))
            # unified PSUM budget: scores 2x[128,1024] (4 banks) + AV pair
            # (2 banks) + scratch for proj/outproj (2 banks) = 8 banks
            spool = ctx.enter_context(tc.tile_pool(name="sps", bufs=2, space="PSUM"))
            apool = ctx.enter_context(tc.tile_pool(name="avs", bufs=2, space="PSUM"))
            gpool = ctx.enter_context(tc.tile_pool(name="gps", bufs=2, space="PSUM"))

            qt_sb = [
                qk_pool.tile([128, T], FR, tag=f"qt{m}", name=f"qt{m}")
                for m in range(4)
            ]
            kt_sb = [
                qk_pool.tile([128, T], FR, tag=f"kt{m}", name=f"kt{m}")
                for m in range(4)
            ]
            vaug = v_pool.tile([128, 16, HL, D + 1], FR, tag="va", name="va")
            wq_sb = [
                wpool.tile([128, KC, 128], FR, tag=f"wq{m}", name=f"wq{m}")
                for m in range(4)
            ]
            wk_sb = [
                wpool.tile([128, KC, 128], FR, tag=f"wk{m}", name=f"wk{m}")
                for m in range(4)
            ]
            wv_sb = wpool.tile([128, KC, 512], FR, tag="wv", name="wv")
            x_ts = [
                xpool.tile([128, KC, 512], FR, tag=f"x{t}", name=f"x{t}")
                for t in range(4)
            ]
            rcF = rpool.tile([128, T], FR, tag="rc", name="rc")
            rsF = rpool.tile([128, T], FR, tag="rs", name="rs")
            wp_sb = c2.tile([128, 4, C], FR, tag="wp", name="wp")
            dm_sb = c2.tile([128, 128], FR, tag="dm", name="dm")
            e_sb = c2.tile([2, 128], FR, tag="es", name="es")

            # ---- input DMAs, split across both HWDGE rings; x0 in halves so
            # the first projection group starts after ~1 MB of stream.
            nc.sync.dma_start(out=wk_sb[0], in_=wkT4[0])
            nc.sync.dma_start(out=wq_sb[0], in_=wqT4[0])
            nc.sync.dma_start(out=x_ts[0][:, 0:4, :], in_=xT4[0][:, 0:4, :])
            nc.sync.dma_start(out=x_ts[0][:, 4:KC, :], in_=xT4[0][:, 4:KC, :])
            nc.sync.dma_start(out=rcF, in_=ropeC)
            nc.sync.dma_start(out=rsF, in_=ropeS2)
            nc.sync.dma_start(out=wv_sb, in_=wvT)
            nc.sync.dma_start(out=dm_sb, in_=dmask)
            nc.sync.dma_start(out=e_sb, in_=esel)
            nc.sync.dma_start(out=x_ts[2], in_=xT4[2])
            nc.sync.dma_start(out=wp_sb, in_=wpT)
            nc.scalar.dma_start(out=vaug[:, :, :, D : D + 1], in_=onesV)
            nc.scalar.dma_start(out=x_ts[1], in_=xT4[1])
            nc.scalar.dma_start(out=wk_sb[1], in_=wkT4[1])
            nc.scalar.dma_start(out=wq_sb[1], in_=wqT4[1])
            nc.scalar.dma_start(out=x_ts[3], in_=xT4[3])
            nc.scalar.dma_start(out=wk_sb[2], in_=wkT4[2])
            nc.scalar.dma_start(out=wq_sb[2], in_=wqT4[2])
            nc.scalar.dma_start(out=wk_sb[3], in_=wkT4[3])
            nc.scalar.dma_start(out=wq_sb[3], in_=wqT4[3])

            # ---- phase-1 emit helpers --------------------------------------
            def proj_group(wsb, t):
                ps = gpool.tile([128, 512], F, tag="g", name="ps")
                for k in range(KC):
                    nc.tensor.matmul(
                        ps,
                        lhsT=wsb[:, k, :],
                        rhs=x_ts[t][:, k, :],
                        start=(k == 0),
                        stop=(k == KC - 1),
                    )
                return ps

            def emit_half(m, wsb, dst):
                """One of Q^T/K^T tiles + rope for head pair m.

                rope: dst = q*C + swap(q*S'') -- both multiplies read the
                projection PSUM; the block-swap DMAs run on the result."""
                usw = fpool.tile([128, T], FR, tag="usw", name="usw")
                uq1 = fpool.tile([128, T], FR, tag="uq1", name="uq1")
                ut2 = fpool.tile([128, T], FR, tag="ut2", name="ut2")
                for t in range(4):
                    sl = slice(512 * t, 512 * (t + 1))
                    ps = proj_group(wsb, t)
                    nc.vector.tensor_mul(usw[:, sl], ps, rsF[:, sl])
                    nc.vector.tensor_mul(uq1[:, sl], ps, rcF[:, sl])
                for a, b in ((0, 32), (32, 0), (64, 96), (96, 64)):
                    nc.gpsimd.dma_start(
                        out=ut2[a : a + 32, :], in_=usw[b : b + 32, :]
                    )
                nc.vector.tensor_add(dst[m], uq1, ut2)

            def emit_pair0_t(t):
                # one t-slice of pair 0 (emitted just before the attention
                # ci-chunk that first needs it, tracking the x DMA stream)
                sl = slice(512 * t, 512 * (t + 1))
                for wsb, dst in ((wk_sb[0], kt_sb), (wq_sb[0], qt_sb)):
                    ps = proj_group(wsb, t)
                    usw = tpool.tile([128, 512], FR, tag="usw4", name="usw")
                    ut2 = tpool.tile([128, 512], FR, tag="ut24", name="ut2")
                    nc.vector.tensor_mul(usw, ps, rsF[:, sl])
                    nc.vector.tensor_mul(dst[0][:, sl], ps, rcF[:, sl])
                    for a, b in ((0, 32), (32, 0), (64, 96), (96, 64)):
                        nc.gpsimd.dma_start(
                            out=ut2[a : a + 32, :], in_=usw[b : b + 32, :]
                        )
                    nc.vector.tensor_add(
                        dst[0][:, sl], dst[0][:, sl], ut2
                    )

            def emit_v(jjs):
                for jj in jjs:
                    t, tt = jj // 4, jj % 4
                    ps = gpool.tile([128, 512], F, tag="g", name="ps")
                    for k in range(KC):
                        nc.tensor.matmul(
                            ps,
                            lhsT=x_ts[t][:, k, 128 * tt : 128 * (tt + 1)],
                            rhs=wv_sb[:, k, :],
                            start=(k == 0),
                            stop=(k == KC - 1),
                        )
                    nc.vector.tensor_copy(
                        out=vaug[:, jj, :, 0:D],
                        in_=ps.rearrange("p (h d) -> p h d", h=HL),
                    )

            yn_t = [[None] * 4 for _ in range(4)]  # [ci][p]

            def emit_outproj_chunk(ci):
                for g in range(8):
                    tt, cc = g % 4, g // 4
                    pr = gpool.tile([128, 512], F, tag="g", name="pr")
                    for p in range(4):
                        nc.tensor.matmul(
                            pr,
                            lhsT=yn_t[ci][p][:, 128 * tt : 128 * (tt + 1)],
                            rhs=wp_sb[:, p, 512 * cc : 512 * (cc + 1)],
                            start=(p == 0),
                            stop=(p == 3),
                        )
                    osb = osbp.tile([128, 512], FR, tag="osb", name="osb")
                    nc.vector.tensor_copy(osb, pr)
                    nc.sync.dma_start(
                        out=out[
                            512 * ci + 128 * tt : 512 * ci + 128 * (tt + 1),
                            512 * cc : 512 * (cc + 1),
                        ],
                        in_=osb,
                    )

            # ---- emission: pair-0 t-slices + V tiles track the x DMA stream,
            # interleaved with p=0's ci chunks; pair p+1's halves are emitted
            # after (p, ci=0) and (p, ci=1) so their rope chains complete
            # mid-attention and only back-fill PE idle slots.

            pending_norm = None  # (p, ci, yraw, d_bf)

            def flush_norm():
                nonlocal pending_norm
                if pending_norm is None:
                    return
                pp, pci, pyraw, pdb = pending_norm
                # broadcast 1/den to 64 rows per head: PE outer product with
                # the [2,128] row-selector (rows 0:64 <- h0, 64:128 <- h1)
                bc = gpool.tile([128, 512], F, tag="g", name="bcp")
                nc.tensor.matmul(bc, lhsT=e_sb, rhs=pdb, start=True, stop=True)
                pynorm = ynp.tile([128, 512], FR, tag="yn", name="yn")
                nc.vector.tensor_mul(pynorm, pyraw, bc)
                yn_t[pci][pp] = pynorm
                pending_norm = None
                if pp == 3:
                    emit_outproj_chunk(pci)

            for p in range(4):
                for ci in range(4):
                    if p == 0:
                        emit_pair0_t(ci)
                        emit_v(range(4 * ci, 4 * ci + 4))
                    o_ps = [
                        apool.tile([128, 512], F, tag="o", name="o") for _ in range(2)
                    ]
                    ntj = 4 * ci + 4
                    for tj in range(ntj):
                        kk = tj - 4 * ci
                        off = 128 * max(kk, 0)
                        s_ps = spool.tile([128, 1024], F, tag="s", name="s")
                        for h in range(2):
                            nc.tensor.matmul(
                                s_ps[:, 512 * h + off : 512 * h + 512],
                                lhsT=kt_sb[p][
                                    64 * h : 64 * h + 64,
                                    128 * tj : 128 * (tj + 1),
                                ],
                                rhs=qt_sb[p][
                                    64 * h : 64 * h + 64,
                                    512 * ci + off : 512 * (ci + 1),
                                ],
                                start=True,
                                stop=True,
                                tile_position=(64 * h, 0),
                            )
                        pt = ppool.tile([128, 1024], FR, tag="pt", name="pt")
                        if kk < 0:
                            nc.scalar.activation(pt, s_ps, EXP, scale=scale)
                        else:
                            s_v = s_ps.rearrange("q (h n) -> q h n", h=2)[:, :, off:]
                            p_v = pt.rearrange("q (h n) -> q h n", h=2)[:, :, off:]
                            nc.scalar.activation(p_v, s_v, EXP, scale=scale)
                            # multiplicative causal mask on the diagonal block,
                            # both heads in one DVE op via stride-0 mid-dim
                            blk = pt.rearrange("q (h n) -> q h n", h=2)[
                                :, :, off : off + 128
                            ]
                            dm_bc = bass.AP(
                                tensor=dm_sb.tensor,
                                offset=dm_sb.offset,
                                ap=[list(dm_sb.ap[0]), [0, 2], list(dm_sb.ap[1])],
                            )
                            nc.vector.tensor_mul(blk, blk, dm_bc)
                        for h in range(2):
                            nc.tensor.matmul(
                                o_ps[h][0 : D + 1, off:512],
                                lhsT=vaug[:, tj, 2 * p + h, :],
                                rhs=pt[:, 512 * h + off : 512 * h + 512],
                                start=(tj == 0),
                                stop=(tj == ntj - 1),
                                skip_group_check=True,
                            )
                    # epilogue: extract O + denominators, reciprocal, broadcast
                    yraw = yrawp.tile([128, 512], F, tag="yraw", name="yraw")
                    ytmp = ytmpp.tile([128, 512], F, tag="ytmp", name="ytmp")
                    d_sb = dpool.tile([128, 1024], F, tag="D", name="D")
                    d_bf = bcpool.tile([2, 512], FR, tag="db", name="db")
                    nc.vector.tensor_copy(yraw[0:65, :], o_ps[0][0:65, :])
                    nc.vector.tensor_copy(ytmp[0:65, :], o_ps[1][0:65, :])
                    nc.sync.dma_start(out=d_sb[0:1, 0:512], in_=yraw[64:65, :])
                    nc.sync.dma_start(out=d_sb[1:2, 0:512], in_=ytmp[64:65, :])
                    nc.sync.dma_start(out=yraw[64:128, :], in_=ytmp[0:64, :])
                    nc.vector.reciprocal_approx_fast(
                        out=d_sb[0:2, 512:1024], in_=d_sb[0:2, 0:512]
                    )
                    nc.vector.tensor_copy(d_bf, d_sb[0:2, 512:1024])
                    flush_norm()
                    pending_norm = (p, ci, yraw, d_bf)
                    if p == 3:
                        # outproj ci gates on this norm -- flush immediately
                        flush_norm()
                    elif ci == 0:
                        emit_half(p + 1, wk_sb[p + 1], kt_sb)
                    elif ci == 1:
                        emit_half(p + 1, wq_sb[p + 1], qt_sb)
            flush_norm()

    nc.compile()
    return nc


def _get_nc(with_bias: bool):
    if with_bias not in _NC_CACHE:
        _NC_CACHE[with_bias] = _build_nc(with_bias)
    return _NC_CACHE[with_bias]


def _rope_tables():
    half = D // 2
    i = np.arange(half, dtype=np.float32)
    expo = (2.0 * i / np.float32(D)).astype(np.float32)
    alpha = (1.0 / (np.float32(10000.0) ** expo)).astype(np.float32)
    ang = (np.arange(T, dtype=np.float32)[:, None] * alpha[None, :]).astype(np.float32)
    cosv = np.cos(ang).astype(np.float32).T  # [32, T]
    sinv = np.sin(ang).astype(np.float32).T
    c64 = np.concatenate([cosv, cosv], axis=0)  # [64, T]
    s64sw = np.concatenate([sinv, -sinv], axis=0)  # swap(S'): S'=[-sin;sin]
    ropeC = np.ascontiguousarray(np.concatenate([c64, c64], axis=0))  # [128, T]
    ropeS2 = np.ascontiguousarray(np.concatenate([s64sw, s64sw], axis=0))
    return (
        ropeC.astype(ml_dtypes.bfloat16),
        ropeS2.astype(ml_dtypes.bfloat16),
    )


def _bf16(a):
    return np.ascontiguousarray(
        np.asarray(a, dtype=np.float32).astype(ml_dtypes.bfloat16)
    )


def _make_in_maps(x, wq, bq, wk, bk, wv, bv, wp, with_bias):
    KC = 9 if with_bias else 8
    ropeC, ropeS2 = _rope_tables()
    dmask_np = np.triu(np.ones((128, 128), np.float32)).astype(ml_dtypes.bfloat16)
    onesV_np = np.ones((128, 16 * HL), dtype=ml_dtypes.bfloat16)
    esel_np = np.zeros((2, 128), np.float32)
    esel_np[0, 0:64] = 1.0
    esel_np[1, 64:128] = 1.0
    esel_np = esel_np.astype(ml_dtypes.bfloat16)

    in_maps = []
    for b in range(B):
        xb = np.ascontiguousarray(x[b].T.astype(np.float32, copy=False))  # [C, T]
        if with_bias:
            aug = np.zeros((KC * 128 - C, T), np.float32)
            aug[0, :] = 1.0
            xb = np.concatenate([xb, aug], axis=0)
        # [KC*128, T] -> [4, 128, KC, 512]
        xb4 = np.ascontiguousarray(
            xb.reshape(KC, 128, 4, 512).transpose(2, 1, 0, 3)
        )
        for g in range(2):
            sl = slice(g * CL, (g + 1) * CL)

            def _prep_qk(w, bias):
                wT = w[sl, :].T.astype(np.float32, copy=False)  # [C, CL]
                if with_bias:
                    npad = KC * 128 - C
                    a = np.zeros((npad, CL), np.float32)
                    a[0, :] = bias[sl].astype(np.float32, copy=False)
                    wT = np.concatenate([wT, a], axis=0)
                # [KC*128, CL] -> [4, 128, KC, 128]
                return np.ascontiguousarray(
                    wT.reshape(KC, 128, 4, 128).transpose(2, 1, 0, 3)
                )

            wq4 = _prep_qk(wq, bq)
            wk4 = _prep_qk(wk, bk)

            wvT_ = wv[sl, :].T.astype(np.float32, copy=False)  # [C, CL]
            if with_bias:
                npad = KC * 128 - C
                a = np.zeros((npad, CL), np.float32)
                a[0, :] = bv[sl].astype(np.float32, copy=False)
                wvT_ = np.concatenate([wvT_, a], axis=0)
            wv3 = np.ascontiguousarray(wvT_.reshape(KC, 128, 512).transpose(1, 0, 2))

            wpTc = wp[:, sl].T.astype(np.float32, copy=False)  # [CL, C]
            wp3 = np.ascontiguousarray(wpTc.reshape(4, 128, C).transpose(1, 0, 2))

            in_maps.append(
                {
                    "xT4": _bf16(xb4),
                    "wqT4": _bf16(wq4),
                    "wkT4": _bf16(wk4),
                    "wvT": _bf16(wv3),
                    "wpT": _bf16(wp3),
                    "onesV": onesV_np,
                    "ropeC": ropeC,
                    "ropeS2": ropeS2,
                    "dmask": dmask_np,
                    "esel": esel_np,
                }
            )
    return in_maps


def _gather(results, bp):
    out = np.empty((B, T, C), dtype=np.float32)
    bp32 = np.asarray(bp, dtype=np.float32)
    for b in range(B):
        out[b] = (
            results[2 * b]["out"].astype(np.float32)
            + results[2 * b + 1]["out"].astype(np.float32)
            + bp32
        )
    return out


def run(x, wq, bq, wk, bk, wv, bv, wp, bp, trace=False, **kw):
    """Build/compile (cached), run on 8 cores, gather. Returns (out, results)."""
    arrs = [np.asarray(a) for a in (x, wq, bq, wk, bk, wv, bv, wp, bp)]
    x, wq, bq, wk, bk, wv, bv, wp, bp = arrs
    with_bias = bool(np.any(bq) or np.any(bk) or np.any(bv))
    nc = _get_nc(with_bias)
    in_maps = _make_in_maps(x, wq, bq, wk, bk, wv, bv, wp, with_bias)
    res = run_bass_kernel_spmd(nc, in_maps, list(range(NCORES)), trace=trace, **kw)
    return _gather(res.results, bp), res


def kernel(x, wq, bq, wk, bk, wv, bv, wp, bp):
    out, _ = run(x, wq, bq, wk, bk, wv, bv, wp, bp)
    return out
